# revision 10
# baseline (speedup 1.0000x reference)
"""Trainium2 Bass kernel for the CP-PINN tensor reconstruction problem.

Computes, for xs (3,320,1) and three per-axis MLP weight stacks:
    f_d = MLP_d(xs[d])            (320, 64)   [tanh MLP: 1->128->128->128->64]
    out[a,b,c] = sum_r f_0[a,r] * f_1[b,r] * f_2[c,r]   ->  (320, 320, 320) f32

Strategy: data-parallel over the output's first axis across 8 NeuronCores
(40 a-points per core, no collectives). The output stream is fp16 (fp16
rounding is ~3e-4 rel-L2, far under the 2e-2 gate), halving the HBM
write floor from ~45.8us to ~22.9us per core. Each core:
  - loads ALL weights/biases with a single host-packed DMA,
  - computes the three MLPs in rank-major f32, interleaved layer-by-layer
    on TensorEngine (matmuls) + ScalarEngine (tanh); final-layer bias-adds
    on VectorE write f32 factor tiles duplicated into both partition
    halves (f0 packed as f0p (128, 20): rows 0-63 = f0[:, a], rows
    64-127 = f0[:, a+20]),
  - Khatri-Rao kr[r, a*N+b] = f0[r,a]*f1[r,b] on the otherwise-idle
    GPSIMD engine (20 dual-half tensor_scalar_mul ops, f32), keeping the
    two PSUM-evacuation engines (VectorE/ScalarE) free for copies,
  - reconstructs its (40*320, 320) slab as 25 quads x 2 two-bank PSUM
    pair-tiles (lo rows / hi rows; 4 tiles in flight for fine-grained
    pipelining): 2 matmuls per tile, one 2-block strided PSUM->fp16-SBUF
    copy per tile (VectorE/ScalarE, rate-balanced ~61/39 assignment),
    staged contiguously per stream and written out with grouped DMAs all
    issued from the otherwise-idle SP sequencer (so no compute engine
    ever stalls behind a waiting dma_start).
"""

import sys

if "/opt/trn_rl_repo" not in sys.path:
    sys.path.insert(0, "/opt/trn_rl_repo")

import numpy as np

import concourse.bacc as bacc
import concourse.mybir as mybir
from concourse import tile
from concourse.bass_utils import run_bass_kernel_spmd

DIMS = 3
N = 320          # points per coordinate axis
R = 64           # CP rank
H = 128          # hidden width
NCORES = 8
NA = N // NCORES          # a-points per core (40)
NROWS = NA * N            # output rows per core (12800)
MCH = 128                 # (a,b)-rows per matmul chunk
NCHUNK = NROWS // MCH     # 100
NPAIR = NCHUNK // 2       # 50 low/high chunk pairs
NQUAD = NPAIR // 2        # 25 two-pair quads
GROUPS_Q = (1, 2, 4, 5, 5, 4, 2, 1, 1)   # quads per output DMA group
assert sum(GROUPS_Q) == NQUAD
GMAX = max(GROUPS_Q)

# Copy-engine assignment per PSUM pair-tile, sequence [lo_q0, hi_q0,
# lo_q1, hi_q1, ...]: 'v' = VectorE (~716ns/copy), 's' = ScalarE
# (~1122ns/copy). Rate-balanced: DVE share 1122/(1122+716) ~ 61%.
_NV = 30
COPY_ENG = tuple(
    'v' if i in {round(j * 50 / _NV) for j in range(_NV)} else 's'
    for i in range(50))

KR_ENGINE = "gpsimd"   # "gpsimd" | "vector"

# Packed-weights column layout (one (128, WCOLS) f32 tensor):
#   [0,384)    w1 (3 x 128 cols)        [384,768)  w2
#   [768,1152) w3 duplicated: per dim 128 cols = [w3 | w3] so ONE f32r
#              matmul writes the factor into both partition halves
#              (f32r matmuls cannot target a PSUM partition offset)
#   [1152,1155) b0 [1155,1158) b1 [1158,1161) b2 [1161,1164) b3 (dup halves)
#   [1164,1548) w0 (row 0 only, 3 x 128 cols)   [1548,2228) packed x (row 0)
W1_OFF, W2_OFF, W3_OFF = 0, 384, 768
B0_OFF, B1_OFF, B2_OFF, B3_OFF = 1152, 1155, 1158, 1161
W0_OFF, WCOLS = 1164, 2228
XP_OFF = 1548
# Packed-x layout (row 0 of wp, from XP_OFF): x0(40) | x1(320) | x2(320)
X0_OFF, X1_OFF, X2_OFF, XCOLS = 0, NA, NA + N, NA + 2 * N

F32 = mybir.dt.float32
F32R = mybir.dt.float32r
F16 = mybir.dt.float16
TANH = mybir.ActivationFunctionType.Tanh

_PROG = None


def _build_program(loop=1, variant="full"):
    """loop>1 wraps the whole compute body in a Tile hardware For_i that
    repeats it `loop` times inside one NEFF launch — benchmarking only."""
    nc = bacc.Bacc("TRN2", target_bir_lowering=False)

    wp = nc.dram_tensor("wp", [H, WCOLS], F32, kind="ExternalInput")
    out = nc.dram_tensor("out", [NROWS, N], F16, kind="ExternalOutput")

    with tile.TileContext(nc) as tc:
        with (
            tc.tile_pool(name="consts", bufs=1) as consts,
            tc.tile_pool(name="work", bufs=2) as work,
            tc.tile_pool(name="stage", bufs=3) as stagep,
            tc.tile_pool(name="ps", bufs=4, space="PSUM") as psp,
        ):
            wp_sb = consts.tile([H, WCOLS], F32)
            nc.sync.dma_start(wp_sb[:], wp[:, :])
            # f32r-rounded copy: everything a matmul consumes (weights and
            # the packed x row) must be *produced* as f32r.
            wp_r = consts.tile([H, WCOLS], F32R)
            nc.vector.tensor_copy(wp_r[:], wp_sb[:])

            import contextlib
            loop_cm = (tc.For_i(0, loop, 1,
                                hint_engines=(mybir.EngineType.PE,))
                       if loop > 1 else contextlib.nullcontext())
            with loop_cm:
                _emit_body(nc, tc, consts, work, stagep, psp,
                           out, wp_sb, wp_r, variant)

    nc.compile()
    return nc


def _tile_copy_views(ps, stg, i):
    """(src, dst) for evacuating pair-tile ps (banks [t | t+1], 320 cols
    each at offsets 0/512) into staging slot i (contiguous 640 cols)."""
    src = ps[:, :].rearrange("p (b x) -> p b x", x=512)[:, :, 0:N]
    dst = stg[:, i * 2 * N:(i + 1) * 2 * N].rearrange("p (b c) -> p b c", c=N)
    return src, dst


def _emit_body(nc, tc, consts, work, stagep, psp, out, wp_sb, wp_r,
               variant="full"):
    outv = out[:, :].rearrange("(m p) c -> p m c", p=MCH)

    warm = work.tile([1, 1], F32, name="warm", tag="warm")
    nc.vector.memset(warm[:], 0.0)
    nc.scalar.activation(warm[:], warm[:], TANH)

    if variant == "empty":
        return

    if variant in ("dma_only", "dma_2ring", "cp_dve", "cp_act"):
        if variant in ("cp_dve", "cp_act"):
            ps0 = psp.tile([MCH, 1024], F32, name="ps0", tag="cps")
            nc.scalar.copy(ps0[:, 0:512], wp_sb[:, 0:512])
            nc.scalar.copy(ps0[:, 512:1024], wp_sb[:, 0:512])
        q = 0
        for gsz in GROUPS_Q:
            stg_lo = stagep.tile([MCH, GMAX * 2 * N], F16, name="stg_lo",
                                 tag="stg_lo")
            stg_hi = stagep.tile([MCH, GMAX * 2 * N], F16, name="stg_hi",
                                 tag="stg_hi")
            if variant in ("dma_only", "dma_2ring"):
                nc.vector.memset(stg_lo[:, 0:1], 1.0)
                nc.vector.memset(stg_hi[:, 0:1], 1.0)
            else:
                eng = (nc.vector.tensor_copy if variant == "cp_dve"
                       else nc.scalar.copy)
                for i in range(gsz):
                    src, dl = _tile_copy_views(ps0, stg_lo, i)
                    _, dh = _tile_copy_views(ps0, stg_hi, i)
                    eng(dl, src)
                    eng(dh, src)
                q += gsz
                continue
            t0 = 2 * q
            lo = stg_lo[:, 0:gsz * 2 * N].rearrange("p (m c) -> p m c", c=N)
            hi = stg_hi[:, 0:gsz * 2 * N].rearrange("p (m c) -> p m c", c=N)
            nc.sync.dma_start(outv[:, t0:t0 + 2 * gsz, :], lo)
            (nc.scalar if variant == "dma_2ring" else nc.sync).dma_start(
                outv[:, NPAIR + t0:NPAIR + t0 + 2 * gsz, :], hi)
            q += gsz
        return

    # f32 factor tiles, duplicated across both partition halves.
    # f0p: rows 0-63 = f0[:, j], rows 64-127 = f0[:, j+20].
    f0p = consts.tile([2 * R, NA // 2], F32)
    f1_sb = consts.tile([2 * R, N], F32)
    f2_sb = consts.tile([2 * R, N], F32R)

    # The three MLPs interleaved layer-by-layer so PE never waits on the
    # ScalarEngine tanh of the same dim (PE executes in program order).
    dims = [(0, X0_OFF, NA), (1, X1_OFF, N), (2, X2_OFF, N)]
    h_cur = {d: wp_r[0:1, XP_OFF + xoff:XP_OFF + xoff + npts]
             for d, xoff, npts in dims}
    w_l0 = wp_r[0:1, :]
    for li, (w_off, b_off, w_ap, wid) in enumerate((
            (W0_OFF, B0_OFF, w_l0, H), (W1_OFF, B1_OFF, wp_r, H))):
        for d, _, npts in dims:
            ps = psp.tile([H, 1024], F32, name=f"ps{li}_{d}", tag="cps")
            nc.tensor.matmul(ps[:, 0:npts],
                             w_ap[:, w_off + d * wid:w_off + (d + 1) * wid],
                             h_cur[d], start=True, stop=True)
            h = work.tile([H, npts], F32R, name=f"h{li}_{d}", tag=f"h_{d}")
            nc.scalar.activation(h[:], ps[:, 0:npts], TANH,
                                 bias=wp_sb[:, b_off + d:b_off + d + 1])
            h_cur[d] = h
    # Last hidden layer + final layer fused per-dim so dim d's factor
    # tile is ready as early as possible (d0/d1 feed the KR stream; d2
    # is only needed by the first CP matmul). Final-layer bias-adds on
    # VectorE (idle during the head; ACT is busy with tanh).
    for d, _, npts in dims:
        ps = psp.tile([H, 1024], F32, name=f"ps2_{d}", tag="cps")
        nc.tensor.matmul(ps[:, 0:npts],
                         wp_r[:, W2_OFF + d * H:W2_OFF + (d + 1) * H],
                         h_cur[d], start=True, stop=True)
        h = work.tile([H, npts], F32R, name=f"h2_{d}", tag=f"h_{d}")
        nc.scalar.activation(h[:], ps[:, 0:npts], TANH,
                             bias=wp_sb[:, B2_OFF + d:B2_OFF + d + 1])
        w3d = wp_r[:, W3_OFF + d * H:W3_OFF + (d + 1) * H]
        psf = psp.tile([2 * R, 1024], F32, name=f"psf_{d}", tag="cps")
        nc.tensor.matmul(psf[:, 0:npts], w3d, h[:], start=True, stop=True)
        b3 = wp_sb[:, B3_OFF + d:B3_OFF + d + 1]
        if d == 0:
            half = NA // 2
            nc.vector.tensor_scalar_add(f0p[0:R, :], psf[0:R, 0:half],
                                        b3[0:R, :])
            nc.vector.tensor_scalar_add(f0p[R:2 * R, :],
                                        psf[R:2 * R, half:NA], b3[R:2 * R, :])
        else:
            f_sb = f1_sb if d == 1 else f2_sb
            nc.vector.tensor_scalar_add(f_sb[:], psf[:, 0:npts], b3)

    if variant == "mlp_only":
        sink = work.tile([2 * R, N], F32, name="sink", tag="sink")
        nc.vector.tensor_copy(sink[:], f2_sb[:])
        nc.vector.tensor_copy(sink[:], f1_sb[:])
        nc.vector.tensor_copy(sink[:, 0:NA // 2], f0p[:])
        return

    # Khatri-Rao: kr[r, a*N + b] = f0[r, a] * f1[r, b], f32, both output
    # halves per op (low partitions: a = j, high: a = j + 20). Runs on
    # GPSIMD so the PSUM-evacuation engines stay free. Emitted
    # just-in-time per quad so the first copies aren't delayed.
    kr_sb = consts.tile([2 * R, NROWS // 2], F32R)
    kr_emitted = 0
    kr_eng = nc.gpsimd if KR_ENGINE == "gpsimd" else nc.vector

    def emit_kr_upto(a_need):
        nonlocal kr_emitted
        while kr_emitted < min(a_need, NA // 2):
            j = kr_emitted
            kr_eng.tensor_scalar_mul(kr_sb[:, j * N:(j + 1) * N],
                                     f1_sb[:, :], f0p[:, j:j + 1])
            kr_emitted += 1

    if variant == "mlp_kr":
        emit_kr_upto(NA // 2)
        return

    # CP reconstruction: 25 quads in tapered DMA groups. Quad q covers
    # chunk pairs t0=2q, 2q+1 as TWO 2-bank PSUM pair-tiles (lo rows /
    # hi rows), 2 matmuls each at column offsets 0/512, one 2-block
    # strided copy per tile -> contiguous fp16 staging per stream, one
    # grouped DMA per stream, all on the SP ring.
    ci = 0
    q = 0
    for gsz in GROUPS_Q:
        stg_lo = stagep.tile([MCH, GMAX * 2 * N], F16, name="stg_lo",
                             tag="stg_lo")
        stg_hi = stagep.tile([MCH, GMAX * 2 * N], F16, name="stg_hi",
                             tag="stg_hi")
        for i in range(gsz):
            t0 = 2 * (q + i)
            emit_kr_upto(-(-((t0 + 4) * MCH) // N))
            ps_lo = psp.tile([MCH, 1024], F32, name="cps_lo", tag="cps")
            ps_hi = psp.tile([MCH, 1024], F32, name="cps_hi", tag="cps")
            for k in (0, 1):
                t = t0 + k
                nc.tensor.matmul(ps_lo[:, k * 512:k * 512 + N],
                                 kr_sb[0:R, t * MCH:(t + 1) * MCH],
                                 f2_sb[0:R, :], start=True, stop=True)
                nc.tensor.matmul(ps_hi[:, k * 512:k * 512 + N],
                                 kr_sb[R:2 * R, t * MCH:(t + 1) * MCH],
                                 f2_sb[R:2 * R, :], start=True, stop=True)
            if variant == "no_copy":
                continue
            src, dst = _tile_copy_views(ps_lo, stg_lo, i)
            (nc.vector.tensor_copy if COPY_ENG[ci] == 'v'
             else nc.scalar.copy)(dst, src)
            src, dst = _tile_copy_views(ps_hi, stg_hi, i)
            (nc.vector.tensor_copy if COPY_ENG[ci + 1] == 'v'
             else nc.scalar.copy)(dst, src)
            ci += 2
        if variant in ("no_copy", "no_dma"):
            q += gsz
            continue
        t0 = 2 * q
        lo = stg_lo[:, 0:gsz * 2 * N].rearrange("p (m c) -> p m c", c=N)
        hi = stg_hi[:, 0:gsz * 2 * N].rearrange("p (m c) -> p m c", c=N)
        nc.sync.dma_start(outv[:, t0:t0 + 2 * gsz, :], lo)
        nc.sync.dma_start(outv[:, NPAIR + t0:NPAIR + t0 + 2 * gsz, :], hi)
        q += gsz


def _get_program():
    global _PROG
    if _PROG is None:
        _PROG = _build_program()
    return _PROG


def _pack_weights(W0, b0, W1, b1, W2, b2, W3, b3):
    wp = np.zeros((H, WCOLS), np.float32)
    for d in range(DIMS):
        wp[:, W1_OFF + d * H:W1_OFF + (d + 1) * H] = W1[d]
        wp[:, W2_OFF + d * H:W2_OFF + (d + 1) * H] = W2[d]
        wp[:, W3_OFF + d * H:W3_OFF + d * H + R] = W3[d]
        wp[:, W3_OFF + d * H + R:W3_OFF + (d + 1) * H] = W3[d]
        wp[:, B0_OFF + d] = b0[d]
        wp[:, B1_OFF + d] = b1[d]
        wp[:, B2_OFF + d] = b2[d]
        wp[0:R, B3_OFF + d] = b3[d]
        wp[R:2 * R, B3_OFF + d] = b3[d]
        wp[0, W0_OFF + d * H:W0_OFF + (d + 1) * H] = W0[d, 0]
    return wp


def _make_in_maps(xs, W0, b0, W1, b1, W2, b2, W3, b3):
    f = lambda x: np.ascontiguousarray(np.asarray(x), dtype=np.float32)
    xs = f(xs)
    wp = _pack_weights(f(W0), f(b0), f(W1), f(b1), f(W2), f(b2), f(W3), f(b3))
    in_maps = []
    for i in range(NCORES):
        w = wp.copy()
        w[0, XP_OFF + X0_OFF:XP_OFF + X0_OFF + NA] = xs[0, i * NA:(i + 1) * NA, 0]
        w[0, XP_OFF + X1_OFF:XP_OFF + X1_OFF + N] = xs[1, :, 0]
        w[0, XP_OFF + X2_OFF:XP_OFF + X2_OFF + N] = xs[2, :, 0]
        in_maps.append({"wp": w})
    return in_maps


def run_spmd(inputs_kwargs, **run_kwargs):
    """Build (cached) program, run on all 8 cores; returns BassKernelResults."""
    nc = _get_program()
    in_maps = _make_in_maps(**inputs_kwargs)
    return run_bass_kernel_spmd(nc, in_maps, core_ids=list(range(NCORES)),
                                **run_kwargs)


def kernel(xs, W0, b0, W1, b1, W2, b2, W3, b3):
    res = run_spmd(dict(xs=xs, W0=W0, b0=b0, W1=W1, b1=b1,
                        W2=W2, b2=b2, W3=W3, b3=b3))
    slabs = [r["out"].astype(np.float32).reshape(NA, N, N)
             for r in res.results]
    return np.concatenate(slabs, axis=0)


# revision 11
# speedup vs baseline: 2.4591x; 2.4591x over previous
"""Trainium2 Bass kernel for the CP-PINN tensor reconstruction problem.

Computes, for xs (3,320,1) and three per-axis MLP weight stacks:
    f_d = MLP_d(xs[d])            (320, 64)   [tanh MLP: 1->128->128->128->64]
    out[a,b,c] = sum_r f_0[a,r] * f_1[b,r] * f_2[c,r]   ->  (320, 320, 320) f32

Strategy: data-parallel over the output's first axis across 8 NeuronCores
(40 a-points per core, no collectives). The output stream is fp16 (fp16
rounding is ~3e-4 rel-L2, far under the 2e-2 gate), halving the HBM
write floor from ~45.8us to ~22.9us per core. Each core:
  - loads ALL weights/biases with a single host-packed DMA,
  - computes the three MLPs in rank-major f32, interleaved layer-by-layer
    on TensorEngine (matmuls) + ScalarEngine (tanh); final-layer bias-adds
    on VectorE write f32 factor tiles duplicated into both partition
    halves (f0 packed as f0p (128, 20): rows 0-63 = f0[:, a], rows
    64-127 = f0[:, a+20]),
  - Khatri-Rao kr[r, a*N+b] = f0[r,a]*f1[r,b] on the otherwise-idle
    GPSIMD engine (20 dual-half tensor_scalar_mul ops, f32), keeping the
    two PSUM-evacuation engines (VectorE/ScalarE) free for copies,
  - reconstructs its (40*320, 320) slab as 25 quads x 2 two-bank PSUM
    pair-tiles (lo rows / hi rows; 4 tiles in flight for fine-grained
    pipelining): 2 matmuls per tile, one 2-block strided PSUM->fp16-SBUF
    copy per tile (VectorE/ScalarE, rate-balanced ~61/39 assignment),
    staged contiguously per stream and written out with grouped DMAs all
    issued from the otherwise-idle SP sequencer (so no compute engine
    ever stalls behind a waiting dma_start).
"""

import sys

if "/opt/trn_rl_repo" not in sys.path:
    sys.path.insert(0, "/opt/trn_rl_repo")

import numpy as np

import concourse.bacc as bacc
import concourse.mybir as mybir
from concourse import tile
from concourse.bass_utils import run_bass_kernel_spmd

DIMS = 3
N = 320          # points per coordinate axis
R = 64           # CP rank
H = 128          # hidden width
NCORES = 8
NA = N // NCORES          # a-points per core (40)
NROWS = NA * N            # output rows per core (12800)
MCH = 128                 # (a,b)-rows per matmul chunk
NCHUNK = NROWS // MCH     # 100
NPAIR = NCHUNK // 2       # 50 low/high chunk pairs
NQUAD = NPAIR // 2        # 25 two-pair quads
GROUPS_Q = (1, 3, 5, 6, 5, 3, 1, 1)   # quads per output DMA group
assert sum(GROUPS_Q) == NQUAD
GMAX = max(GROUPS_Q)

# Copy-engine assignment per quad (one 4-block copy per quad): 'v' =
# VectorE, 's' = ScalarE. DVE also carries the KR stream + final adds;
# ACT carries the MLP head + the hi-stream DMA issues.
_NV = 13
COPY_ENG = tuple(
    'v' if i in {round(j * NQUAD / _NV) for j in range(_NV)} else 's'
    for i in range(NQUAD))

KR_ENGINE = "vector"   # "gpsimd" | "vector" (gpsimd: ~6us/op Q7 dispatch - unusable)

# Packed-weights column layout (one (128, WCOLS) f32 tensor):
#   [0,384)    w1 (3 x 128 cols)        [384,768)  w2
#   [768,1152) w3 duplicated: per dim 128 cols = [w3 | w3] so ONE f32r
#              matmul writes the factor into both partition halves
#              (f32r matmuls cannot target a PSUM partition offset)
#   [1152,1155) b0 [1155,1158) b1 [1158,1161) b2 [1161,1164) b3 (dup halves)
#   [1164,1548) w0 (row 0 only, 3 x 128 cols)   [1548,2228) packed x (row 0)
W1_OFF, W2_OFF, W3_OFF = 0, 384, 768
B0_OFF, B1_OFF, B2_OFF, B3_OFF = 1152, 1155, 1158, 1161
W0_OFF, WCOLS = 1164, 2228
XP_OFF = 1548
# Packed-x layout (row 0 of wp, from XP_OFF): x0(40) | x1(320) | x2(320)
X0_OFF, X1_OFF, X2_OFF, XCOLS = 0, NA, NA + N, NA + 2 * N

F32 = mybir.dt.float32
F32R = mybir.dt.float32r
F16 = mybir.dt.float16
TANH = mybir.ActivationFunctionType.Tanh

_PROG = None


def _build_program(loop=1, variant="full"):
    """loop>1 wraps the whole compute body in a Tile hardware For_i that
    repeats it `loop` times inside one NEFF launch — benchmarking only."""
    nc = bacc.Bacc("TRN2", target_bir_lowering=False)

    wp = nc.dram_tensor("wp", [H, WCOLS], F32, kind="ExternalInput")
    out = nc.dram_tensor("out", [NROWS, N], F16, kind="ExternalOutput")

    with tile.TileContext(nc) as tc:
        with (
            tc.tile_pool(name="consts", bufs=1) as consts,
            tc.tile_pool(name="work", bufs=2) as work,
            tc.tile_pool(name="stage", bufs=3) as stagep,
            tc.tile_pool(name="ps", bufs=2, space="PSUM") as psp,
        ):
            wp_sb = consts.tile([H, WCOLS], F32)
            nc.sync.dma_start(wp_sb[:], wp[:, :])
            # f32r-rounded copy: everything a matmul consumes (weights and
            # the packed x row) must be *produced* as f32r.
            wp_r = consts.tile([H, WCOLS], F32R)
            nc.vector.tensor_copy(wp_r[:], wp_sb[:])

            import contextlib
            loop_cm = (tc.For_i(0, loop, 1,
                                hint_engines=(mybir.EngineType.PE,))
                       if loop > 1 else contextlib.nullcontext())
            with loop_cm:
                _emit_body(nc, tc, consts, work, stagep, psp,
                           out, wp_sb, wp_r, variant)

    nc.compile()
    return nc


def _quad_copy_views(ps, stg, i):
    """(src, dst) for evacuating quad tile ps (4 banks [lo_t | lo_t+1 |
    hi_t | hi_t+1], 320 cols each at offsets 0/512/1024/1536) into group
    staging slot i: lo pairs at [i*640, (i+1)*640), hi pairs at
    [GMAX*640 + i*640, ...) — both DMA streams read contiguously."""
    src = ps[:, :].rearrange("p (s b x) -> p s b x", s=2, x=512)[:, :, :, 0:N]
    dst = (stg[:, :].rearrange("p (s r) -> p s r", s=2)
           [:, :, i * 2 * N:(i + 1) * 2 * N]
           .rearrange("p s (b c) -> p s b c", c=N))
    return src, dst


def _emit_body(nc, tc, consts, work, stagep, psp, out, wp_sb, wp_r,
               variant="full"):
    outv = out[:, :].rearrange("(m p) c -> p m c", p=MCH)

    warm = work.tile([1, 1], F32, name="warm", tag="warm")
    nc.vector.memset(warm[:], 0.0)
    nc.scalar.activation(warm[:], warm[:], TANH)

    if variant == "empty":
        return

    if variant in ("dma_only", "dma_2ring", "cp_dve", "cp_act"):
        if variant in ("cp_dve", "cp_act"):
            ps0 = psp.tile([MCH, 2048], F32, name="ps0", tag="cps")
            for j in range(4):
                nc.scalar.copy(ps0[:, j * 512:(j + 1) * 512], wp_sb[:, 0:512])
        q = 0
        for gsz in GROUPS_Q:
            stg = stagep.tile([MCH, 2 * GMAX * 2 * N], F16, name="stg",
                              tag="stg")
            if variant in ("dma_only", "dma_2ring"):
                nc.vector.memset(stg[:, 0:1], 1.0)
            else:
                eng = (nc.vector.tensor_copy if variant == "cp_dve"
                       else nc.scalar.copy)
                for i in range(gsz):
                    src, dst = _quad_copy_views(ps0, stg, i)
                    eng(dst, src)
                q += gsz
                continue
            t0 = 2 * q
            sv = stg[:, :].rearrange("p (s r) -> p s r", s=2)
            lo = sv[:, 0, 0:gsz * 2 * N].rearrange("p (m c) -> p m c", c=N)
            hi = sv[:, 1, 0:gsz * 2 * N].rearrange("p (m c) -> p m c", c=N)
            nc.sync.dma_start(outv[:, t0:t0 + 2 * gsz, :], lo)
            (nc.scalar if variant == "dma_2ring" else nc.sync).dma_start(
                outv[:, NPAIR + t0:NPAIR + t0 + 2 * gsz, :], hi)
            q += gsz
        return

    # f32 factor tiles, duplicated across both partition halves.
    # f0p: rows 0-63 = f0[:, j], rows 64-127 = f0[:, j+20].
    f0p = consts.tile([2 * R, NA // 2], F32)
    f1_sb = consts.tile([2 * R, N], F32)
    f2_sb = consts.tile([2 * R, N], F32R)

    # The three MLPs interleaved layer-by-layer so PE never waits on the
    # ScalarEngine tanh of the same dim (PE executes in program order).
    dims = [(0, X0_OFF, NA), (1, X1_OFF, N), (2, X2_OFF, N)]
    h_cur = {d: wp_r[0:1, XP_OFF + xoff:XP_OFF + xoff + npts]
             for d, xoff, npts in dims}
    w_l0 = wp_r[0:1, :]
    for li, (w_off, b_off, w_ap, wid) in enumerate((
            (W0_OFF, B0_OFF, w_l0, H), (W1_OFF, B1_OFF, wp_r, H))):
        for d, _, npts in dims:
            ps = psp.tile([H, 2048], F32, name=f"ps{li}_{d}", tag="cps")
            nc.tensor.matmul(ps[:, 0:npts],
                             w_ap[:, w_off + d * wid:w_off + (d + 1) * wid],
                             h_cur[d], start=True, stop=True)
            h = work.tile([H, npts], F32R, name=f"h{li}_{d}", tag=f"h_{d}")
            nc.scalar.activation(h[:], ps[:, 0:npts], TANH,
                                 bias=wp_sb[:, b_off + d:b_off + d + 1])
            h_cur[d] = h
    # Last hidden layer + final layer fused per-dim so dim d's factor
    # tile is ready as early as possible (d0/d1 feed the KR stream; d2
    # is only needed by the first CP matmul). Final-layer bias-adds on
    # VectorE (idle during the head; ACT is busy with tanh).
    for d, _, npts in dims:
        ps = psp.tile([H, 2048], F32, name=f"ps2_{d}", tag="cps")
        nc.tensor.matmul(ps[:, 0:npts],
                         wp_r[:, W2_OFF + d * H:W2_OFF + (d + 1) * H],
                         h_cur[d], start=True, stop=True)
        h = work.tile([H, npts], F32R, name=f"h2_{d}", tag=f"h_{d}")
        nc.scalar.activation(h[:], ps[:, 0:npts], TANH,
                             bias=wp_sb[:, B2_OFF + d:B2_OFF + d + 1])
        w3d = wp_r[:, W3_OFF + d * H:W3_OFF + (d + 1) * H]
        psf = psp.tile([2 * R, 2048], F32, name=f"psf_{d}", tag="cps")
        nc.tensor.matmul(psf[:, 0:npts], w3d, h[:], start=True, stop=True)
        b3 = wp_sb[:, B3_OFF + d:B3_OFF + d + 1]
        if d == 0:
            half = NA // 2
            nc.vector.tensor_scalar_add(f0p[0:R, :], psf[0:R, 0:half],
                                        b3[0:R, :])
            nc.vector.tensor_scalar_add(f0p[R:2 * R, :],
                                        psf[R:2 * R, half:NA], b3[R:2 * R, :])
        else:
            f_sb = f1_sb if d == 1 else f2_sb
            nc.vector.tensor_scalar_add(f_sb[:], psf[:, 0:npts], b3)

    if variant == "mlp_only":
        sink = work.tile([2 * R, N], F32, name="sink", tag="sink")
        nc.vector.tensor_copy(sink[:], f2_sb[:])
        nc.vector.tensor_copy(sink[:], f1_sb[:])
        nc.vector.tensor_copy(sink[:, 0:NA // 2], f0p[:])
        return

    # Khatri-Rao: kr[r, a*N + b] = f0[r, a] * f1[r, b], f32r, both
    # output halves per op (low partitions: a = j, high: a = j + 20).
    # Emitted just-in-time per quad so the first copies aren't delayed.
    kr_sb = consts.tile([2 * R, NROWS // 2], F32R)
    kr_emitted = 0
    kr_eng = nc.gpsimd if KR_ENGINE == "gpsimd" else nc.vector

    def emit_kr_upto(a_need):
        nonlocal kr_emitted
        while kr_emitted < min(a_need, NA // 2):
            j = kr_emitted
            kr_eng.tensor_scalar_mul(kr_sb[:, j * N:(j + 1) * N],
                                     f1_sb[:, :], f0p[:, j:j + 1])
            kr_emitted += 1

    if variant == "mlp_kr":
        emit_kr_upto(NA // 2)
        return

    # CP reconstruction: 25 quads in tapered DMA groups. Quad q covers
    # chunk pairs t0=2q, 2q+1 as ONE 4-bank PSUM tile [lo_t0 | lo_t1 |
    # hi_t0 | hi_t1], 4 matmuls, one 4-block strided copy into group
    # staging (lo region | hi region, each contiguous). Per group: lo
    # DMA on the SP ring; hi DMA on the ScalarE ring, but EMITTED one
    # quad into the next group so it never stalls ACT's in-order copy
    # queue while waiting for the group's last copy.
    pending_hi = None

    def flush_hi():
        nonlocal pending_hi
        if pending_hi is not None:
            nc.scalar.dma_start(*pending_hi)
            pending_hi = None

    q = 0
    for gsz in GROUPS_Q:
        stg = stagep.tile([MCH, 2 * GMAX * 2 * N], F16, name="stg",
                          tag="stg")
        for i in range(gsz):
            t0 = 2 * (q + i)
            emit_kr_upto(-(-((t0 + 4) * MCH) // N))
            ps = psp.tile([MCH, 2048], F32, name="cps", tag="cps")
            for k in (0, 1):
                t = t0 + k
                nc.tensor.matmul(ps[:, k * 512:k * 512 + N],
                                 kr_sb[0:R, t * MCH:(t + 1) * MCH],
                                 f2_sb[0:R, :], start=True, stop=True)
                nc.tensor.matmul(ps[:, 1024 + k * 512:1024 + k * 512 + N],
                                 kr_sb[R:2 * R, t * MCH:(t + 1) * MCH],
                                 f2_sb[R:2 * R, :], start=True, stop=True)
            if variant == "no_copy":
                continue
            src, dst = _quad_copy_views(ps, stg, i)
            (nc.vector.tensor_copy if COPY_ENG[q + i] == 'v'
             else nc.scalar.copy)(dst, src)
            if i == 0 and variant not in ("no_dma",):
                flush_hi()
        if variant in ("no_copy", "no_dma"):
            q += gsz
            continue
        t0 = 2 * q
        sv = stg[:, :].rearrange("p (s r) -> p s r", s=2)
        lo = sv[:, 0, 0:gsz * 2 * N].rearrange("p (m c) -> p m c", c=N)
        hi = sv[:, 1, 0:gsz * 2 * N].rearrange("p (m c) -> p m c", c=N)
        nc.sync.dma_start(outv[:, t0:t0 + 2 * gsz, :], lo)
        pending_hi = (outv[:, NPAIR + t0:NPAIR + t0 + 2 * gsz, :], hi)
        q += gsz
    flush_hi()


def _get_program():
    global _PROG
    if _PROG is None:
        _PROG = _build_program()
    return _PROG


def _pack_weights(W0, b0, W1, b1, W2, b2, W3, b3):
    wp = np.zeros((H, WCOLS), np.float32)
    for d in range(DIMS):
        wp[:, W1_OFF + d * H:W1_OFF + (d + 1) * H] = W1[d]
        wp[:, W2_OFF + d * H:W2_OFF + (d + 1) * H] = W2[d]
        wp[:, W3_OFF + d * H:W3_OFF + d * H + R] = W3[d]
        wp[:, W3_OFF + d * H + R:W3_OFF + (d + 1) * H] = W3[d]
        wp[:, B0_OFF + d] = b0[d]
        wp[:, B1_OFF + d] = b1[d]
        wp[:, B2_OFF + d] = b2[d]
        wp[0:R, B3_OFF + d] = b3[d]
        wp[R:2 * R, B3_OFF + d] = b3[d]
        wp[0, W0_OFF + d * H:W0_OFF + (d + 1) * H] = W0[d, 0]
    return wp


def _make_in_maps(xs, W0, b0, W1, b1, W2, b2, W3, b3):
    f = lambda x: np.ascontiguousarray(np.asarray(x), dtype=np.float32)
    xs = f(xs)
    wp = _pack_weights(f(W0), f(b0), f(W1), f(b1), f(W2), f(b2), f(W3), f(b3))
    in_maps = []
    for i in range(NCORES):
        w = wp.copy()
        w[0, XP_OFF + X0_OFF:XP_OFF + X0_OFF + NA] = xs[0, i * NA:(i + 1) * NA, 0]
        w[0, XP_OFF + X1_OFF:XP_OFF + X1_OFF + N] = xs[1, :, 0]
        w[0, XP_OFF + X2_OFF:XP_OFF + X2_OFF + N] = xs[2, :, 0]
        in_maps.append({"wp": w})
    return in_maps


def run_spmd(inputs_kwargs, **run_kwargs):
    """Build (cached) program, run on all 8 cores; returns BassKernelResults."""
    nc = _get_program()
    in_maps = _make_in_maps(**inputs_kwargs)
    return run_bass_kernel_spmd(nc, in_maps, core_ids=list(range(NCORES)),
                                **run_kwargs)


def kernel(xs, W0, b0, W1, b1, W2, b2, W3, b3):
    res = run_spmd(dict(xs=xs, W0=W0, b0=b0, W1=W1, b1=b1,
                        W2=W2, b2=b2, W3=W3, b3=b3))
    slabs = [r["out"].astype(np.float32).reshape(NA, N, N)
             for r in res.results]
    return np.concatenate(slabs, axis=0)
